# revision 57
# baseline (speedup 1.0000x reference)
"""Multi-head attention (16 heads, L=2312, E=1024) on 8 trn2 NeuronCores.

Sharding: tensor-parallel over heads — each core computes 2 heads' full
attention (QKV proj + RoPE + softmax(QK^T)V), merged per query block.

Key structure:
- Score matmuls contract over only 64 head dims, so the two heads run
  CONCURRENTLY as a 64x128 row-tiled pair on the PE subarrays (T0 uses SBUF
  partitions 0-63 = head 0, T8 uses 64-127 = head 1) — ~1.4x score speedup
  and no zero-padded per-head Q variants.
- Tokens are processed in a stripe-permuted order (host-side permutation):
  each core's 304-token output shard is split into 4 stripes (64/64/64/112)
  laid out so that kernel columns [0,512) hold every shard's stripe 0, etc.
  The context re-shard then runs as FOUR quarter AllToAlls fired as the
  norm front passes kernel cols 512/1024/1536/2432 — the first three fully
  overlap attention compute, only the last is exposed.
- The context matmuls are row-tiled pairs too (key tiles split 64/64), so
  the whole phase-B PE stream stays in one tiling mode; the two partial
  sums are combined by the DVE copy that the softmax divide needs anyway.
- Transpose-free softmax divide: denominator rows are spread across
  partitions by a tiny SBUF->SBUF reshape DMA, reciprocated on the DVE in
  one 8-col op, DMAed to DRAM and broadcast-read back across partitions
  (DMA partition-broadcast works from DRAM), and applied with two DVE
  multiplies.

Numerics: bf16 operands with fp32 PSUM accumulation + fp32 softmax.

Self-contained: all shapes hardcoded; takes full unsharded inputs.
"""
import os
import numpy as np
import ml_dtypes

KBISECT = set(os.environ.get("KBISECT", "").split(","))

import concourse.bacc as bacc
import concourse.tile as tile
from concourse import mybir
from concourse.bass_utils import run_bass_kernel_spmd
from concourse.masks import make_identity

N_CORES = 8
L = 2312           # valid sequence length
LP = 2432          # padded to 19*128
NK = LP // 128     # 19 key tiles
E = 1024
KE = E // 128      # 8 contraction tiles over embed dim
SHARD = LP // N_CORES  # 304 tokens of output per core
F32 = mybir.dt.float32
BF16 = mybir.dt.bfloat16
SCALE = 0.125      # 1/sqrt(64)

# Stripe-permuted kernel-column geometry: kernel quarter q holds stripe q of
# every shard back-to-back; shard j's stripe widths are QW.
QW = (64, 64, 64, 112)       # stripe widths
QOFF = (0, 512, 1024, 1536)  # kernel col where quarter q starts
QTOK = (0, 64, 128, 192)     # within-shard token offset of stripe q
QEND = (512, 1024, 1536, 2432)
# kernel pad columns (original tokens >= 2312): shard 7 tails of stripes 2,3
PADRUNS = ((1528, 8), (2320, 112))
# valid kernel cols are [0,1528) + [1536,2320); QKV blocks cover exactly those
NBLK = [(0, 256), (256, 256), (512, 512), (1024, 504), (1536, 512), (2048, 272)]
# attention query blocks (same runs); DONE = norm coverage incl pad cols
LQB = [(0, 512), (512, 512), (1024, 504), (1536, 512), (2048, 272)]
LQB_DONE = (512, 1024, 1536, 2048, 2432)

_NC_CACHE = {}


def _build():
    if "nc" in _NC_CACHE:
        return _NC_CACHE["nc"]
    nc = bacc.Bacc(
        "TRN2",
        target_bir_lowering=False,
        debug=False,
        enable_asserts=False,
        num_devices=N_CORES,
    )
    XCOLS = KE * L  # 18496
    xT = nc.dram_tensor("xT", [128, XCOLS], BF16, kind="ExternalInput").ap()
    wT = nc.dram_tensor("wT", [128, 3 * KE * 128], BF16, kind="ExternalInput").ap()
    bqkv = nc.dram_tensor("bqkv", [128, 3], F32, kind="ExternalInput").ap()
    cosT = nc.dram_tensor("cosT", [128, LP], BF16, kind="ExternalInput").ap()
    sinT = nc.dram_tensor("sinT", [128, LP], BF16, kind="ExternalInput").ap()
    mskT = nc.dram_tensor("mskT", [128, NK], F32, kind="ExternalInput").ap()
    pwT = nc.dram_tensor("pwT", [E, E], BF16, kind="ExternalInput").ap()
    pb = nc.dram_tensor("pb", [128, KE], F32, kind="ExternalInput").ap()
    perm = nc.dram_tensor("perm", [128, 128], BF16, kind="ExternalInput").ap()
    outT = nc.dram_tensor("outT", [E, SHARD], BF16, kind="ExternalOutput").ap()

    with tile.TileContext(nc) as tc:
        with (
            tc.tile_pool(name="const", bufs=1) as cpool,
            tc.tile_pool(name="dram", bufs=1, space="DRAM") as dpool,
            tc.tile_pool(name="qkv", bufs=1) as qkvpool,
            tc.tile_pool(name="vaugp", bufs=1) as vaugpool,
            tc.tile_pool(name="ctxp", bufs=1) as ctxpool,
            tc.tile_pool(name="psb", bufs=5) as pspool,
            tc.tile_pool(name="ct", bufs=3) as ctpool,
            tc.tile_pool(name="rp", bufs=3) as rpool,
            tc.tile_pool(name="rrd", bufs=3, space="DRAM") as rrdpool,
            tc.tile_pool(name="pw_ag", bufs=1) as pwpool,
        ):
            identb = cpool.tile([128, 128], BF16)
            pbias = cpool.tile([128, KE], F32)
            mask_sb = cpool.tile([128, NK], F32)
            perm_sb = cpool.tile([128, 128], BF16)

            Q = qkvpool.tile([128, LP], BF16)
            K = qkvpool.tile([128, LP], BF16)
            V = qkvpool.tile([128, LP], BF16)
            vaug = vaugpool.tile([128, NK, 130], BF16)
            ctxTn = ctxpool.tile([128, LP], BF16)
            cc_in = [dpool.tile([N_CORES, 128, QW[q]], BF16, name=f"cci{q}")
                     for q in range(4)]
            cc_out = [dpool.tile([N_CORES, 128, QW[q]], BF16, name=f"cco{q}")
                      for q in range(4)]

            # PSUM pool for the per-(block,head) context accumulators — tags
            # pc0/pc1/repl, one bank each (ring-1: a block's accumulator is
            # recycled as soon as the previous block's norm copies it out).
            # Spans phases A and B; closed before phase C needs all 8 banks.
            spA_cm = tc.tile_pool(name="ps_s0", bufs=2, space="PSUM")
            spA = spA_cm.__enter__()
            psc_cm = tc.tile_pool(name="ps_c", bufs=1, space="PSUM")
            psc = psc_cm.__enter__()

            # ---------------- Phase A: QKV projection + RoPE + V transpose ----
            with (
                tc.tile_pool(name="xw", bufs=1) as xwpool,
                tc.tile_pool(name="ropet", bufs=4) as rtp,
                tc.tile_pool(name="ps_a", bufs=2, space="PSUM") as psa,
            ):
                x_sb = xwpool.tile([128, XCOLS], BF16)
                w_sb = xwpool.tile([128, 3 * KE * 128], BF16)
                b_sb = xwpool.tile([128, 3], F32)
                cos_sb = xwpool.tile([128, LP], BF16)
                sin_sb = xwpool.tile([128, LP], BF16)
                xoff = {}
                off = 0
                for (n0, nw) in NBLK:
                    xoff[n0] = off
                    off += KE * nw

                # first x chunk rides the otherwise-idle scalar queue so it
                # lands in parallel with the w sections on sync
                nc.scalar.dma_start(x_sb[:, :KE * NBLK[0][1]], xT[:, :KE * NBLK[0][1]])
                for sec in range(3):
                    nc.sync.dma_start(
                        w_sb[:, 1024 * sec:1024 * (sec + 1)],
                        wT[:, 1024 * sec:1024 * (sec + 1)],
                    )
                for (n0, nw) in NBLK[1:]:
                    o = xoff[n0]
                    nc.sync.dma_start(
                        x_sb[:, o:o + KE * nw], xT[:, o:o + KE * nw]
                    )
                nc.scalar.dma_start(cos_sb[:, 0:512], cosT[:, 0:512])
                nc.scalar.dma_start(sin_sb[:, 0:512], sinT[:, 0:512])
                nc.scalar.dma_start(cos_sb[:, 512:1536], cosT[:, 512:1536])
                nc.scalar.dma_start(sin_sb[:, 512:1536], sinT[:, 512:1536])
                nc.scalar.dma_start(cos_sb[:, 1536:2320], cosT[:, 1536:2320])
                nc.scalar.dma_start(sin_sb[:, 1536:2320], sinT[:, 1536:2320])
                nc.gpsimd.dma_start(perm_sb[:], perm)
                nc.gpsimd.dma_start(b_sb[:], bqkv)
                nc.gpsimd.dma_start(mask_sb[:], mskT)
                nc.gpsimd.dma_start(pbias[:], pb)

                make_identity(nc, identb[:])

                # static zero regions: kernel pad cols of K/V (masked keys
                # must be finite) and of the shipped context
                for (p0, pw_) in PADRUNS:
                    nc.vector.memset(K[:, p0:p0 + pw_], 0.0)
                    nc.vector.memset(V[:, p0:p0 + pw_], 0.0)
                    nc.vector.memset(ctxTn[:, p0:p0 + pw_], 0.0)
                mview = mask_sb[:].rearrange("p (t o) -> p t o", o=1)
                nc.vector.tensor_copy(vaug[:, :, 64:65], mview)
                nc.vector.tensor_copy(vaug[:, :, 129:130], mview)

                def rope_chunk(T, n0, nw):
                    # rotate T[:, n0:n0+nw] in place; the 32-half swap within
                    # each head is a permutation matmul on PE. swp shares the
                    # phase-A PSUM ring with the QKV accumulators.
                    swp = psa.tile([128, 512], F32, tag="psa", name=f"swp_{T.name}_{n0}")
                    nc.tensor.matmul(swp[:, :nw], perm_sb[:], T[:, n0:n0 + nw])
                    sw = rtp.tile([128, 512], BF16, tag="swap", name=f"sw_{T.name}_{n0}")
                    tmp = rtp.tile([128, 512], BF16, tag="tmp", name=f"tmp_{T.name}_{n0}")
                    nc.vector.tensor_mul(tmp[:, :nw], T[:, n0:n0 + nw], cos_sb[:, n0:n0 + nw])
                    nc.vector.tensor_mul(sw[:, :nw], swp[:, :nw], sin_sb[:, n0:n0 + nw])
                    nc.vector.tensor_add(T[:, n0:n0 + nw], tmp[:, :nw], sw[:, :nw])

                def score_pair(SP, t, q0, qw, phase="B"):
                    # both heads' scores as a concurrent 64x128 row-tiled
                    # pair: T0 contracts SBUF partitions 0-63 (head 0), T8
                    # contracts 64-127 (head 1); separate PSUM banks
                    if ("notile" in KBISECT or f"notile{phase}" in KBISECT):  # bisect
                        nc.tensor.matmul(
                            SP[:, 0, :qw], K[:, 128 * t:128 * (t + 1)],
                            Q[:, q0:q0 + qw],
                        )
                        nc.tensor.matmul(
                            SP[:, 1, :qw], K[:, 128 * t:128 * (t + 1)],
                            Q[:, q0:q0 + qw],
                        )
                        return
                    nc.tensor.matmul(
                        SP[:, 0, :qw], K[0:64, 128 * t:128 * (t + 1)],
                        Q[0:64, q0:q0 + qw],
                    )
                    nc.tensor.matmul(
                        SP[:, 1, :qw], K[64:128, 128 * t:128 * (t + 1)],
                        Q[64:128, q0:q0 + qw],
                    )

                def vaug_chunk(n0, nw):
                    for t in range(n0 // 128, (n0 + nw + 127) // 128):
                        tp = psa.tile([128, 128], BF16, tag="psa", name="vtp")
                        nc.tensor.transpose(tp[:], V[:, 128 * t:128 * (t + 1)], identb[:])
                        nc.vector.tensor_scalar_mul(
                            vaug[:, t, 0:64], tp[:, 0:64], mask_sb[:, t:t + 1]
                        )
                        nc.vector.tensor_scalar_mul(
                            vaug[:, t, 65:129], tp[:, 64:128], mask_sb[:, t:t + 1]
                        )

                # attention for block 0 (both heads, full 512-col items)
                # interleaves into phase A between QKV sections
                PC_A = {}            # h -> psum accumulator
                att0_items = []      # key tile index
                att0_pend = None     # (t, PSb) awaiting ctx

                def att0_ctx(pend):
                    t, pb_ = pend
                    for h in range(2):
                        if h not in PC_A:
                            PC_A[h] = psc.tile(
                                [128, 512], F32, tag=f"pc{h}", name=f"pcA{h}"
                            )
                        nc.tensor.matmul(
                            PC_A[h][0:65, :],
                            vaug[:, t, 65 * h:65 * h + 65],
                            pb_[:, h, :],
                            start=(t == 0),
                            stop=(t == NK - 1),
                        )

                def att0_step():
                    nonlocal att0_pend
                    if not att0_items:
                        return
                    t = att0_items.pop(0)
                    SP = spA.tile([128, 2, 512], F32, tag="sp0", name="sp0")
                    PSb = pspool.tile([128, 2, 512], BF16, tag="psb", name="psb")
                    score_pair(SP, t, 0, 512, phase="A")
                    nc.scalar.activation(
                        PSb[:], SP[:],
                        mybir.ActivationFunctionType.Exp, scale=SCALE,
                    )
                    if att0_pend is not None:
                        att0_ctx(att0_pend)
                    att0_pend = (t, PSb)

                # K-tile availability as rope coverage grows; items need the
                # whole 512-col query block 0 roped (cov >= 512)
                avail_tiles = [0]

                def att0_avail(cov):
                    if cov < 512:
                        return
                    # the last valid col is 2320; tile 18's tail is memset
                    # pad, so the final block unlocks all NK tiles
                    nt = NK if cov >= 2320 else cov // 128
                    att0_items.extend(range(avail_tiles[0], nt))
                    avail_tiles[0] = nt

                outs = [Q, K, V]
                for (n0, nw) in NBLK:
                    for m in range(3):
                        ps = psa.tile([128, 512], F32, tag="psa", name="qkvps")
                        for k in range(KE):
                            nc.tensor.matmul(
                                ps[:, :nw],
                                w_sb[:, 1024 * m + 128 * k:1024 * m + 128 * k + 128],
                                x_sb[:, xoff[n0] + nw * k:xoff[n0] + nw * k + nw],
                                start=(k == 0),
                                stop=(k == KE - 1),
                            )
                        nc.vector.tensor_scalar_add(
                            outs[m][:, n0:n0 + nw], ps[:, :nw], b_sb[:, m:m + 1]
                        )
                        if m < 2:
                            rope_chunk(outs[m], n0, nw)
                        else:
                            vaug_chunk(n0, nw)
                        att0_step()
                        att0_step()
                        att0_step()
                    att0_avail(n0 + nw)
                att0_left = list(att0_items)
                att0_items.clear()

            # ---------------- Phase B: merged-head attention ------------------
            if True:
                pw_sb = pwpool.tile([128, KE, E], BF16)
                pwr = pwT.rearrange("(k p) e -> p k e", p=128)
                nc.sync.dma_start(pw_sb[:, 0:4, :], pwr[:, 0:4, :])
                nc.sync.dma_start(pw_sb[:, 4:8, :], pwr[:, 4:8, :])
                ag = pwpool.tile([128, KE, SHARD], BF16)
                osb = pwpool.tile([128, KE, SHARD], BF16)

                if True:
                    norm_q = []
                    cur_norm = [None]
                    cc_next = [0]
                    pend = []        # (t, PSb, bst) ctx groups trailing scores
                    psx = [None]     # ctx-partials pool, opened once psc closes
                    pssl = [None]    # score-tile pool, opened once spA closes

                    def norm_step():
                        while norm_q and not cur_norm[0] and norm_q[0]["atomic"]:
                            e = norm_q.pop(0)
                            for s_fn in e["subs"]:
                                s_fn()
                        if not cur_norm[0] and norm_q:
                            cur_norm[0] = norm_q.pop(0)
                        e = cur_norm[0]
                        if e:
                            e["subs"].pop(0)()
                            if not e["subs"]:
                                cur_norm[0] = None

                    def ship_quarter(q):
                        # ship all 8 shard-stripes of quarter q and fire its
                        # AllToAll; the re-shard DMA lands the result in ag
                        w = QW[q]
                        for j in range(N_CORES):
                            eng = nc.gpsimd if j % 2 == 0 else nc.sync
                            eng.dma_start(
                                cc_in[q][j],
                                ctxTn[:, QOFF[q] + w * j:QOFF[q] + w * (j + 1)],
                            )
                        if "nocc" in KBISECT:  # bisect: skip the collectives
                            nc.gpsimd.dma_start(
                                ag[:, :, QTOK[q]:QTOK[q] + w],
                                cc_in[q][:].rearrange("k d w -> d k w"),
                            )
                            return
                        nc.gpsimd.collective_compute(
                            "AllToAll",
                            mybir.AluOpType.bypass,
                            replica_groups=[list(range(N_CORES))],
                            ins=[cc_in[q].opt()],
                            outs=[cc_out[q].opt()],
                        )
                        # NOTE: the ag re-shard DMA is deferred to after the
                        # attention pass — its trigger instruction blocks the
                        # issuing sequencer until the collective lands, which
                        # would stall every later DMA on that queue

                    def reshard_quarter(q):
                        if "nocc" in KBISECT:
                            return
                        w = QW[q]
                        ccr = cc_out[q][:].rearrange("k d w -> d k w")
                        half = N_CORES // 2
                        nc.gpsimd.dma_start(
                            ag[:, 0:half, QTOK[q]:QTOK[q] + w], ccr[:, 0:half, :]
                        )
                        nc.sync.dma_start(
                            ag[:, half:KE, QTOK[q]:QTOK[q] + w], ccr[:, half:KE, :]
                        )

                    def norm_subs(lq0, lqw, done, bst):
                        # transpose-free softmax divide for both heads; also
                        # combines the two row-tiled ctx partial sums (or
                        # plain-copies block 0's full-contraction result)
                        state = {}
                        pq = lqw // 4  # partitions used by the reshape DMAs

                        def s_copy():
                            CT0 = ctpool.tile([65, 512], BF16, tag="ct0", name="ct0")
                            CT1 = ctpool.tile([65, 512], BF16, tag="ct1", name="ct1")
                            if "PCb0" in bst:
                                # DVE reads at most one PSUM operand: stage
                                # the T8 partials through SBUF
                                TB0 = ctpool.tile([65, 512], BF16, tag="tb0", name="tb0")
                                TB1 = ctpool.tile([65, 512], BF16, tag="tb1", name="tb1")
                                nc.vector.tensor_copy(
                                    TB0[:, :lqw], bst["PCb0"][0:65, :lqw])
                                nc.vector.tensor_copy(
                                    TB1[:, :lqw], bst["PCb1"][0:65, :lqw])
                                nc.vector.tensor_add(
                                    CT0[:, :lqw], bst["PCa0"][0:65, :lqw],
                                    TB0[:, :lqw])
                                nc.vector.tensor_add(
                                    CT1[:, :lqw], bst["PCa1"][0:65, :lqw],
                                    TB1[:, :lqw])
                            else:
                                nc.vector.tensor_copy(
                                    CT0[:, :lqw], bst["PC0"][0:65, :lqw])
                                nc.vector.tensor_copy(
                                    CT1[:, :lqw], bst["PC1"][0:65, :lqw])
                            state["CT0"] = CT0
                            state["CT1"] = CT1

                        def s_recip():
                            D1 = rpool.tile([128, 8], BF16, tag="d1", name="d1")
                            nc.scalar.dma_start(
                                D1[0:pq, 0:4], state["CT0"][64:65, :lqw])
                            nc.sync.dma_start(
                                D1[0:pq, 4:8], state["CT1"][64:65, :lqw])
                            R8 = rpool.tile([128, 8], BF16, tag="r8", name="r8")
                            with nc.allow_low_precision(
                                reason="bf16 recip of a bf16 denominator; "
                                "matches baseline numerics"
                            ):
                                nc.vector.reciprocal(
                                    R8[0:pq, :], D1[0:pq, :])
                            rr2d = rrdpool.tile([2, 512], BF16, tag="rr2d", name="rr2d")
                            nc.scalar.dma_start(rr2d[0:1, :lqw], R8[0:pq, 0:4])
                            nc.sync.dma_start(rr2d[1:2, :lqw], R8[0:pq, 4:8])
                            state["rr2d"] = rr2d

                        def s_bcast():
                            # partition-broadcast works with a DRAM source;
                            # two base-0 tiles (DVE needs equal input bases)
                            RB0 = ctpool.tile([64, 512], BF16, tag="rb0", name="rb0")
                            RB1 = ctpool.tile([64, 512], BF16, tag="rb1", name="rb1")
                            rr = state["rr2d"]
                            nc.scalar.dma_start(
                                RB0[:, :lqw],
                                rr[0:1, :lqw].to_broadcast((64, lqw)))
                            nc.sync.dma_start(
                                RB1[:, :lqw],
                                rr[1:2, :lqw].to_broadcast((64, lqw)))
                            state["RB0"] = RB0
                            state["RB1"] = RB1

                        def s_mul():
                            nc.vector.tensor_mul(
                                ctxTn[0:64, lq0:lq0 + lqw],
                                state["CT0"][0:64, :lqw],
                                state["RB0"][:, :lqw],
                            )
                            nc.vector.tensor_mul(
                                ctxTn[64:128, lq0:lq0 + lqw],
                                state["CT1"][0:64, :lqw],
                                state["RB1"][:, :lqw],
                            )
                            while cc_next[0] < 4 and done >= QEND[cc_next[0]]:
                                ship_quarter(cc_next[0])
                                cc_next[0] += 1

                        return [s_copy, s_recip, s_bcast, s_mul]

                    def flush_pend(n_keep):
                        while len(pend) > n_keep:
                            t, pb_, bst = pend.pop(0)
                            if bst["PCa0"] is None:
                                for nm in ("PCa0", "PCb0", "PCa1", "PCb1"):
                                    bst[nm] = psx[0].tile(
                                        [128, 512], F32, tag=nm,
                                        name=f"{nm}_{bst['lq0']}",
                                    )
                            # row-tiled ctx pairs: T0 sums key rows 0-63 into
                            # PCa, T8 sums rows 64-127 into PCb (separate
                            # banks); the norm's DVE copy adds the partials
                            for h in range(2):
                                nc.tensor.matmul(
                                    bst[f"PCa{h}"][0:65, :bst["lqw"]],
                                    vaug[0:64, t, 65 * h:65 * h + 65],
                                    pb_[0:64, h, :bst["lqw"]],
                                    start=(t == 0),
                                    stop=(t == NK - 1),
                                )
                                nc.tensor.matmul(
                                    bst[f"PCb{h}"][0:65, :bst["lqw"]],
                                    vaug[64:128, t, 65 * h:65 * h + 65],
                                    pb_[64:128, h, :bst["lqw"]],
                                    start=(t == 0),
                                    stop=(t == NK - 1),
                                )

                    def attention_pass(blocks, tighten_tail=False):
                        for bi, (lq0, lqw) in enumerate(blocks):
                            is_last = tighten_tail and (lq0, lqw) == blocks[-1]
                            bst = {"PCa0": None, "PCb0": None, "PCa1": None,
                                   "PCb1": None, "lq0": lq0, "lqw": lqw}
                            sbs = [list(range(g, min(g + 2, NK)))
                                   for g in range(0, NK, 2)]
                            for gi, tl in enumerate(sbs):
                                for t in tl:
                                    SP = pssl[0].tile([128, 2, 512], F32, tag="sp", name="sp")
                                    score_pair(SP, t, lq0, lqw)
                                    PSb = pspool.tile([128, 2, 512], BF16, tag="psb", name="psb")
                                    nc.scalar.activation(
                                        PSb[:, :, :lqw], SP[:, :, :lqw],
                                        mybir.ActivationFunctionType.Exp,
                                        scale=SCALE,
                                    )
                                    pend.append((t, PSb, bst))
                                if gi >= 2:
                                    norm_step()
                                # keep>=4 before the first norm drip so a PC
                                # ring-slot reuse never precedes the s_copy
                                # that frees it
                                flush_pend(1 if (is_last and gi >= 8) else 4)
                            di = LQB.index((lq0, lqw))
                            norm_q.append({
                                "subs": norm_subs(lq0, lqw, LQB_DONE[di], bst),
                                "atomic": False,
                            })

                    # drain leftover phase-A items with phase-B score slots
                    for t in att0_left:
                        SP = spA.tile([128, 2, 512], F32, tag="sp0", name="sp0")
                        PSb = pspool.tile([128, 2, 512], BF16, tag="psb", name="psb")
                        score_pair(SP, t, 0, 512)
                        nc.scalar.activation(
                            PSb[:], SP[:],
                            mybir.ActivationFunctionType.Exp, scale=SCALE,
                        )
                        if att0_pend is not None:
                            att0_ctx(att0_pend)
                        att0_pend = (t, PSb)
                    if att0_pend is not None:
                        att0_ctx(att0_pend)
                        att0_pend = None

                    # block 0 came from phase A: queue its norm and start its
                    # DVE/DMA chain before block 1's score stream
                    norm_q.append({
                        "subs": norm_subs(
                            0, 512, QEND[0],
                            {"PC0": PC_A[0], "PC1": PC_A[1]},
                        ),
                        "atomic": False,
                    })
                    norm_step()  # s_copy consumes PC_A -> psc can close,
                    norm_step()  # freeing its 2 banks for the ctx partials
                    psc_cm.__exit__(None, None, None)
                    spA_cm.__exit__(None, None, None)
                    pss_cm = tc.tile_pool(name="ps_s", bufs=2, space="PSUM")
                    pssl[0] = pss_cm.__enter__()
                    psx_cm = tc.tile_pool(name="ps_x", bufs=1, space="PSUM")
                    psx[0] = psx_cm.__enter__()
                    attention_pass(LQB[1:], tighten_tail=True)
                    flush_pend(0)
                    while norm_q or cur_norm[0]:
                        norm_step()
                    psx_cm.__exit__(None, None, None)
                    pss_cm.__exit__(None, None, None)
                    # land the AllToAll results in SBUF: quarters 0-2 have
                    # long arrived (instant), only quarter 3's trigger waits
                    for q in range(4):
                        reshard_quarter(q)

                # ------------ Phase C: output projection ----------------------
                outTr = outT.rearrange("(k p) n -> p k n", p=128)
                with tc.tile_pool(name="ps_o", bufs=1, space="PSUM") as pso:
                    pos = [
                        pso.tile([128, SHARD], F32, tag=f"po{mE}", name=f"po{mE}")
                        for mE in range(KE)
                    ]
                    # stripes 0-2 (ag cols 0:192) depend only on AllToAlls
                    # 0-2 — the PE churns through them while AllToAll 3 is
                    # still in flight
                    for mE in range(KE):
                        for k in range(KE):
                            nc.tensor.matmul(
                                pos[mE][:, 0:192],
                                pw_sb[:, k, 128 * mE:128 * (mE + 1)],
                                ag[:, k, 0:192],
                                start=(k == 0),
                                stop=(k == KE - 1),
                            )
                    # stripe 3: mE-major so each output chunk's bias-add and
                    # store overlap the remaining chunks' matmuls
                    for mE in range(KE):
                        for k in range(KE):
                            nc.tensor.matmul(
                                pos[mE][:, 192:304],
                                pw_sb[:, k, 128 * mE:128 * (mE + 1)],
                                ag[:, k, 192:304],
                                start=(k == 0),
                                stop=(k == KE - 1),
                            )
                        nc.vector.tensor_scalar_add(
                            osb[:, mE, :], pos[mE][:], pbias[:, mE:mE + 1]
                        )
                        eng = nc.sync if mE % 2 == 0 else nc.gpsimd
                        eng.dma_start(outTr[:, mE, :], osb[:, mE, :])

    nc.compile()
    _NC_CACHE["nc"] = nc
    return nc


def _sigma():
    # kernel col -> original (padded) token index
    s = np.empty(LP, np.int64)
    for j in range(N_CORES):
        for q in range(4):
            s[QOFF[q] + QW[q] * j:QOFF[q] + QW[q] * (j + 1)] = (
                SHARD * j + QTOK[q] + np.arange(QW[q])
            )
    return s


def _prep_inputs(x, key_padding_mask, qkv_w, qkv_b, proj_w, proj_b, freqs_cos, freqs_sin):
    bf = ml_dtypes.bfloat16
    x = np.ascontiguousarray(np.asarray(x, np.float32))
    qkv_w = np.asarray(qkv_w, np.float32)
    qkv_b = np.asarray(qkv_b, np.float32)
    proj_w = np.asarray(proj_w, np.float32)
    proj_b = np.asarray(proj_b, np.float32)
    fc = np.asarray(freqs_cos, np.float32)  # [2304, 64]
    fs = np.asarray(freqs_sin, np.float32)
    mask = np.asarray(key_padding_mask)

    sig = _sigma()

    # chunk-major x in stripe-permuted token order: per NBLK block a
    # contiguous [128, KE*nw] slab with column order (k, n)
    xTf = x.T.astype(bf)  # [E, L]
    xH = np.concatenate(
        [
            xTf[:, sig[n0:n0 + nw]].reshape(KE, 128, nw).transpose(1, 0, 2).reshape(128, KE * nw)
            for (n0, nw) in NBLK
        ],
        axis=1,
    )
    xH = np.ascontiguousarray(xH)

    # rope tables + mask in kernel (permuted) token order
    valid = sig < L
    rot = valid & (sig >= 8)
    cosT = np.ones((64, LP), np.float32)
    cosT[:, rot] = fc.T[:, sig[rot] - 8]
    cos2 = np.concatenate([cosT, cosT], axis=0).astype(bf)

    sinT = np.zeros((64, LP), np.float32)
    sinT[:, rot] = fs.T[:, sig[rot] - 8]
    sinT[:32, :] *= -1.0  # sign of -x2 half folded into sin table
    sin2 = np.concatenate([sinT, sinT], axis=0).astype(bf)

    maskf = np.zeros((LP,), np.float32)
    maskf[valid] = mask.astype(np.float32)[sig[valid]]
    mskT = np.ascontiguousarray(maskf.reshape(NK, 128).T)  # [128, NK]

    # proj_w rows are consumed in natural head order (the quarter AllToAlls
    # deliver source cores' 128-row blocks in core order = head order)
    pwT = np.ascontiguousarray(proj_w.T).astype(bf)  # [d, e]
    permM = np.zeros((128, 128), np.float32)  # lhsT: permM[k, m]=1 iff k==swap(m)
    for m128 in range(128):
        swp = m128 + 32 if (m128 % 64) < 32 else m128 - 32
        permM[swp, m128] = 1.0
    permM = permM.astype(bf)
    pb2 = np.ascontiguousarray(proj_b.reshape(KE, 128).T)  # [128, KE]

    in_maps = []
    for c in range(N_CORES):
        h0, h1 = 2 * c, 2 * c + 1
        rows = []
        bias_rows = []
        for sec in range(3):  # q, k, v sections of qkv_w
            for h in (h0, h1):
                sl = slice(1024 * sec + 64 * h, 1024 * sec + 64 * h + 64)
                rows.append(qkv_w[sl])
                bias_rows.append(qkv_b[sl])
        Wc = np.concatenate(rows, axis=0)           # [384, 1024]
        bc = np.concatenate(bias_rows, axis=0)      # [384]
        WcT = Wc.T.astype(bf)  # [1024, 384]
        wH = np.ascontiguousarray(
            WcT.reshape(KE, 128, 3, 128).transpose(1, 2, 0, 3).reshape(128, 3 * KE * 128)
        )
        in_maps.append({
            "xT": xH,
            "wT": wH,
            "bqkv": np.ascontiguousarray(bc.reshape(3, 128).T),
            "cosT": cos2,
            "sinT": sin2,
            "mskT": mskT,
            "pwT": pwT,
            "pb": pb2,
            "perm": permM,
        })
    return in_maps


def _run(in_maps, trace=False):
    nc = _build()
    return run_bass_kernel_spmd(
        nc, in_maps, core_ids=list(range(N_CORES)), trace=trace
    )


def kernel(x, key_padding_mask, qkv_w, qkv_b, proj_w, proj_b, freqs_cos, freqs_sin):
    in_maps = _prep_inputs(
        x, key_padding_mask, qkv_w, qkv_b, proj_w, proj_b, freqs_cos, freqs_sin
    )
    res = _run(in_maps, trace=False)
    outT_full = np.concatenate(
        [res.results[c]["outT"] for c in range(N_CORES)], axis=1
    )  # [E, LP]; shard c's columns are original tokens [304c, 304c+304)
    return np.ascontiguousarray(outT_full[:, :L].T).astype(np.float32)


# revision 58
# speedup vs baseline: 1.0713x; 1.0713x over previous
"""Multi-head attention (16 heads, L=2312, E=1024) on 8 trn2 NeuronCores.

Sharding: tensor-parallel over heads — each core computes 2 heads' full
attention (QKV proj + RoPE + softmax(QK^T)V), merged per query block.

Key structure:
- Score matmuls contract over only 64 head dims, so the two heads run
  CONCURRENTLY as a 64x128 row-tiled pair on the PE subarrays (T0 uses SBUF
  partitions 0-63 = head 0, T8 uses 64-127 = head 1) — ~1.4x score speedup
  and no zero-padded per-head Q variants.
- Tokens are processed in a stripe-permuted order (host-side permutation):
  each core's 304-token output shard is split into 4 stripes (64/64/64/112)
  laid out so that kernel columns [0,512) hold every shard's stripe 0, etc.
  The context re-shard then runs as FOUR quarter AllToAlls fired as the
  norm front passes kernel cols 512/1024/1536/2432 — the first three fully
  overlap attention compute, only the last is exposed.
- The context matmuls are row-tiled pairs too (key tiles split 64/64), so
  the whole phase-B PE stream stays in one tiling mode; the two partial
  sums are combined by the DVE copy that the softmax divide needs anyway.
- Transpose-free softmax divide: denominator rows are spread across
  partitions by a tiny SBUF->SBUF reshape DMA, reciprocated on the DVE in
  one 8-col op, DMAed to DRAM and broadcast-read back across partitions
  (DMA partition-broadcast works from DRAM), and applied with two DVE
  multiplies.

Numerics: bf16 operands with fp32 PSUM accumulation + fp32 softmax.

Self-contained: all shapes hardcoded; takes full unsharded inputs.
"""
import os
import numpy as np
import ml_dtypes

KBISECT = set(os.environ.get("KBISECT", "").split(","))

import concourse.bacc as bacc
import concourse.tile as tile
from concourse import mybir
from concourse.bass_utils import run_bass_kernel_spmd
from concourse.masks import make_identity

N_CORES = 8
L = 2312           # valid sequence length
LP = 2432          # padded to 19*128
NK = LP // 128     # 19 key tiles
E = 1024
KE = E // 128      # 8 contraction tiles over embed dim
SHARD = LP // N_CORES  # 304 tokens of output per core
F32 = mybir.dt.float32
BF16 = mybir.dt.bfloat16
SCALE = 0.125      # 1/sqrt(64)

# Stripe-permuted kernel-column geometry: kernel quarter q holds stripe q of
# every shard back-to-back; shard j's stripe widths are QW.
QW = (64, 64, 64, 112)       # stripe widths
QOFF = (0, 512, 1024, 1536)  # kernel col where quarter q starts
QTOK = (0, 64, 128, 192)     # within-shard token offset of stripe q
QEND = (512, 1024, 1536, 2432)
# kernel pad columns (original tokens >= 2312): shard 7 tails of stripes 2,3
PADRUNS = ((1528, 8), (2320, 112))
# valid kernel cols are [0,1528) + [1536,2320); QKV blocks cover exactly those
NBLK = [(0, 256), (256, 256), (512, 512), (1024, 504), (1536, 512), (2048, 272)]
# attention query blocks (same runs); DONE = norm coverage incl pad cols
LQB = [(0, 512), (512, 512), (1024, 504), (1536, 512), (2048, 272)]
LQB_DONE = (512, 1024, 1536, 2048, 2432)

_NC_CACHE = {}


def _build():
    if "nc" in _NC_CACHE:
        return _NC_CACHE["nc"]
    nc = bacc.Bacc(
        "TRN2",
        target_bir_lowering=False,
        debug=False,
        enable_asserts=False,
        num_devices=N_CORES,
    )
    XCOLS = KE * L  # 18496
    xT = nc.dram_tensor("xT", [128, XCOLS], BF16, kind="ExternalInput").ap()
    wT = nc.dram_tensor("wT", [128, 3 * KE * 128], BF16, kind="ExternalInput").ap()
    bqkv = nc.dram_tensor("bqkv", [128, 3], F32, kind="ExternalInput").ap()
    cosT = nc.dram_tensor("cosT", [128, LP], BF16, kind="ExternalInput").ap()
    sinT = nc.dram_tensor("sinT", [128, LP], BF16, kind="ExternalInput").ap()
    mskT = nc.dram_tensor("mskT", [128, NK], F32, kind="ExternalInput").ap()
    pwT = nc.dram_tensor("pwT", [E, E], BF16, kind="ExternalInput").ap()
    pb = nc.dram_tensor("pb", [128, KE], F32, kind="ExternalInput").ap()
    perm = nc.dram_tensor("perm", [128, 128], BF16, kind="ExternalInput").ap()
    outT = nc.dram_tensor("outT", [E, SHARD], BF16, kind="ExternalOutput").ap()

    with tile.TileContext(nc) as tc:
        with (
            tc.tile_pool(name="const", bufs=1) as cpool,
            tc.tile_pool(name="dram", bufs=1, space="DRAM") as dpool,
            tc.tile_pool(name="qkv", bufs=1) as qkvpool,
            tc.tile_pool(name="vaugp", bufs=1) as vaugpool,
            tc.tile_pool(name="ctxp", bufs=1) as ctxpool,
            tc.tile_pool(name="psb", bufs=5) as pspool,
            tc.tile_pool(name="ct", bufs=3) as ctpool,
            tc.tile_pool(name="rp", bufs=3) as rpool,
            tc.tile_pool(name="rrd", bufs=3, space="DRAM") as rrdpool,
            tc.tile_pool(name="pw_ag", bufs=1) as pwpool,
        ):
            identb = cpool.tile([128, 128], BF16)
            pbias = cpool.tile([128, KE], F32)
            mask_sb = cpool.tile([128, NK], F32)
            perm_sb = cpool.tile([128, 128], BF16)

            Q = qkvpool.tile([128, LP], BF16)
            K = qkvpool.tile([128, LP], BF16)
            V = qkvpool.tile([128, LP], BF16)
            vaug = vaugpool.tile([128, NK, 130], BF16)
            ctxTn = ctxpool.tile([128, LP], BF16)
            cc_in = [dpool.tile([N_CORES, 128, QW[q]], BF16, name=f"cci{q}")
                     for q in range(4)]
            cc_out = [dpool.tile([N_CORES, 128, QW[q]], BF16, name=f"cco{q}")
                      for q in range(4)]

            # PSUM pool for the per-(block,head) context accumulators — tags
            # pc0/pc1/repl, one bank each (ring-1: a block's accumulator is
            # recycled as soon as the previous block's norm copies it out).
            # Spans phases A and B; closed before phase C needs all 8 banks.
            spA_cm = tc.tile_pool(name="ps_s0", bufs=2, space="PSUM")
            spA = spA_cm.__enter__()
            psc_cm = tc.tile_pool(name="ps_c", bufs=1, space="PSUM")
            psc = psc_cm.__enter__()

            # ---------------- Phase A: QKV projection + RoPE + V transpose ----
            with (
                tc.tile_pool(name="xw", bufs=1) as xwpool,
                tc.tile_pool(name="ropet", bufs=4) as rtp,
                tc.tile_pool(name="ps_a", bufs=2, space="PSUM") as psa,
            ):
                x_sb = xwpool.tile([128, XCOLS], BF16)
                w_sb = xwpool.tile([128, 3 * KE * 128], BF16)
                b_sb = xwpool.tile([128, 3], F32)
                cos_sb = xwpool.tile([128, LP], BF16)
                sin_sb = xwpool.tile([128, LP], BF16)
                xoff = {}
                off = 0
                for (n0, nw) in NBLK:
                    xoff[n0] = off
                    off += KE * nw

                # first x chunk rides the otherwise-idle scalar queue so it
                # lands in parallel with the w sections on sync
                nc.scalar.dma_start(x_sb[:, :KE * NBLK[0][1]], xT[:, :KE * NBLK[0][1]])
                for sec in range(3):
                    nc.sync.dma_start(
                        w_sb[:, 1024 * sec:1024 * (sec + 1)],
                        wT[:, 1024 * sec:1024 * (sec + 1)],
                    )
                for (n0, nw) in NBLK[1:]:
                    o = xoff[n0]
                    nc.sync.dma_start(
                        x_sb[:, o:o + KE * nw], xT[:, o:o + KE * nw]
                    )
                nc.scalar.dma_start(cos_sb[:, 0:512], cosT[:, 0:512])
                nc.scalar.dma_start(sin_sb[:, 0:512], sinT[:, 0:512])
                nc.scalar.dma_start(cos_sb[:, 512:1536], cosT[:, 512:1536])
                nc.scalar.dma_start(sin_sb[:, 512:1536], sinT[:, 512:1536])
                nc.scalar.dma_start(cos_sb[:, 1536:2320], cosT[:, 1536:2320])
                nc.scalar.dma_start(sin_sb[:, 1536:2320], sinT[:, 1536:2320])
                nc.gpsimd.dma_start(perm_sb[:], perm)
                nc.gpsimd.dma_start(b_sb[:], bqkv)
                nc.gpsimd.dma_start(mask_sb[:], mskT)
                nc.gpsimd.dma_start(pbias[:], pb)

                make_identity(nc, identb[:])

                # static zero regions: kernel pad cols of K/V (masked keys
                # must be finite) and of the shipped context
                for (p0, pw_) in PADRUNS:
                    nc.vector.memset(K[:, p0:p0 + pw_], 0.0)
                    nc.vector.memset(V[:, p0:p0 + pw_], 0.0)
                    nc.vector.memset(ctxTn[:, p0:p0 + pw_], 0.0)
                mview = mask_sb[:].rearrange("p (t o) -> p t o", o=1)
                nc.vector.tensor_copy(vaug[:, :, 64:65], mview)
                nc.vector.tensor_copy(vaug[:, :, 129:130], mview)

                def rope_chunk(T, n0, nw):
                    # rotate T[:, n0:n0+nw] in place; the 32-half swap within
                    # each head is a permutation matmul on PE. swp shares the
                    # phase-A PSUM ring with the QKV accumulators.
                    swp = psa.tile([128, 512], F32, tag="psa", name=f"swp_{T.name}_{n0}")
                    nc.tensor.matmul(swp[:, :nw], perm_sb[:], T[:, n0:n0 + nw])
                    sw = rtp.tile([128, 512], BF16, tag="swap", name=f"sw_{T.name}_{n0}")
                    tmp = rtp.tile([128, 512], BF16, tag="tmp", name=f"tmp_{T.name}_{n0}")
                    nc.vector.tensor_mul(tmp[:, :nw], T[:, n0:n0 + nw], cos_sb[:, n0:n0 + nw])
                    nc.vector.tensor_mul(sw[:, :nw], swp[:, :nw], sin_sb[:, n0:n0 + nw])
                    nc.vector.tensor_add(T[:, n0:n0 + nw], tmp[:, :nw], sw[:, :nw])

                def score_pair(SP, t, q0, qw, phase="B"):
                    # both heads' scores as a concurrent 64x128 row-tiled
                    # pair: T0 contracts SBUF partitions 0-63 (head 0), T8
                    # contracts 64-127 (head 1); separate PSUM banks
                    if ("notile" in KBISECT or f"notile{phase}" in KBISECT):  # bisect
                        nc.tensor.matmul(
                            SP[:, 0, :qw], K[:, 128 * t:128 * (t + 1)],
                            Q[:, q0:q0 + qw],
                        )
                        nc.tensor.matmul(
                            SP[:, 1, :qw], K[:, 128 * t:128 * (t + 1)],
                            Q[:, q0:q0 + qw],
                        )
                        return
                    nc.tensor.matmul(
                        SP[:, 0, :qw], K[0:64, 128 * t:128 * (t + 1)],
                        Q[0:64, q0:q0 + qw],
                    )
                    nc.tensor.matmul(
                        SP[:, 1, :qw], K[64:128, 128 * t:128 * (t + 1)],
                        Q[64:128, q0:q0 + qw],
                    )

                def vaug_chunk(n0, nw):
                    for t in range(n0 // 128, (n0 + nw + 127) // 128):
                        tp = psa.tile([128, 128], BF16, tag="psa", name="vtp")
                        nc.tensor.transpose(tp[:], V[:, 128 * t:128 * (t + 1)], identb[:])
                        nc.vector.tensor_scalar_mul(
                            vaug[:, t, 0:64], tp[:, 0:64], mask_sb[:, t:t + 1]
                        )
                        nc.vector.tensor_scalar_mul(
                            vaug[:, t, 65:129], tp[:, 64:128], mask_sb[:, t:t + 1]
                        )

                # attention for block 0 (both heads, full 512-col items)
                # interleaves into phase A between QKV sections
                PC_A = {}            # h -> psum accumulator
                att0_items = []      # key tile index
                att0_pend = None     # (t, PSb) awaiting ctx

                def att0_ctx(pend):
                    t, pb_ = pend
                    for h in range(2):
                        if h not in PC_A:
                            PC_A[h] = psc.tile(
                                [128, 512], F32, tag=f"pc{h}", name=f"pcA{h}"
                            )
                        nc.tensor.matmul(
                            PC_A[h][0:65, :],
                            vaug[:, t, 65 * h:65 * h + 65],
                            pb_[:, h, :],
                            start=(t == 0),
                            stop=(t == NK - 1),
                        )

                def att0_step():
                    nonlocal att0_pend
                    if not att0_items:
                        return
                    t = att0_items.pop(0)
                    SP = spA.tile([128, 2, 512], F32, tag="sp0", name="sp0")
                    PSb = pspool.tile([128, 2, 512], BF16, tag="psb", name="psb")
                    score_pair(SP, t, 0, 512, phase="A")
                    nc.scalar.activation(
                        PSb[:], SP[:],
                        mybir.ActivationFunctionType.Exp, scale=SCALE,
                    )
                    if att0_pend is not None:
                        att0_ctx(att0_pend)
                    att0_pend = (t, PSb)

                # K-tile availability as rope coverage grows; items need the
                # whole 512-col query block 0 roped (cov >= 512)
                avail_tiles = [0]

                def att0_avail(cov):
                    if cov < 512:
                        return
                    # the last valid col is 2320; tile 18's tail is memset
                    # pad, so the final block unlocks all NK tiles
                    nt = NK if cov >= 2320 else cov // 128
                    att0_items.extend(range(avail_tiles[0], nt))
                    avail_tiles[0] = nt

                outs = [Q, K, V]
                for (n0, nw) in NBLK:
                    for m in range(3):
                        ps = psa.tile([128, 512], F32, tag="psa", name="qkvps")
                        for k in range(KE):
                            nc.tensor.matmul(
                                ps[:, :nw],
                                w_sb[:, 1024 * m + 128 * k:1024 * m + 128 * k + 128],
                                x_sb[:, xoff[n0] + nw * k:xoff[n0] + nw * k + nw],
                                start=(k == 0),
                                stop=(k == KE - 1),
                            )
                        nc.vector.tensor_scalar_add(
                            outs[m][:, n0:n0 + nw], ps[:, :nw], b_sb[:, m:m + 1]
                        )
                        if m < 2:
                            rope_chunk(outs[m], n0, nw)
                        else:
                            vaug_chunk(n0, nw)
                        att0_step()
                        att0_step()
                        att0_step()
                    att0_avail(n0 + nw)
                att0_left = list(att0_items)
                att0_items.clear()

            # ---------------- Phase B: merged-head attention ------------------
            if True:
                pw_sb = pwpool.tile([128, KE, E], BF16)
                pwr = pwT.rearrange("(k p) e -> p k e", p=128)
                nc.sync.dma_start(pw_sb[:, 0:4, :], pwr[:, 0:4, :])
                nc.sync.dma_start(pw_sb[:, 4:8, :], pwr[:, 4:8, :])
                ag = pwpool.tile([128, KE, SHARD], BF16)
                osb = pwpool.tile([128, KE, SHARD], BF16)

                if True:
                    norm_q = []
                    cur_norm = [None]
                    cc_next = [0]
                    pend = []        # (t, PSb, bst) ctx groups trailing scores
                    psx = [None]     # ctx-partials pool, opened once psc closes
                    pssl = [None]    # score-tile pool, opened once spA closes

                    def norm_step():
                        while norm_q and not cur_norm[0] and norm_q[0]["atomic"]:
                            e = norm_q.pop(0)
                            for s_fn in e["subs"]:
                                s_fn()
                        if not cur_norm[0] and norm_q:
                            cur_norm[0] = norm_q.pop(0)
                        e = cur_norm[0]
                        if e:
                            e["subs"].pop(0)()
                            if not e["subs"]:
                                cur_norm[0] = None

                    def ship_quarter(q):
                        # ship all 8 shard-stripes of quarter q and fire its
                        # AllToAll; the re-shard DMA lands the result in ag
                        w = QW[q]
                        for j in range(N_CORES):
                            eng = nc.gpsimd if j % 2 == 0 else nc.sync
                            eng.dma_start(
                                cc_in[q][j],
                                ctxTn[:, QOFF[q] + w * j:QOFF[q] + w * (j + 1)],
                            )
                        if "nocc" in KBISECT:  # bisect: skip the collectives
                            nc.gpsimd.dma_start(
                                ag[:, :, QTOK[q]:QTOK[q] + w],
                                cc_in[q][:].rearrange("k d w -> d k w"),
                            )
                            return
                        nc.gpsimd.collective_compute(
                            "AllToAll",
                            mybir.AluOpType.bypass,
                            replica_groups=[list(range(N_CORES))],
                            ins=[cc_in[q].opt()],
                            outs=[cc_out[q].opt()],
                        )
                        # NOTE: the ag re-shard DMA is deferred to after the
                        # attention pass — its trigger instruction blocks the
                        # issuing sequencer until the collective lands, which
                        # would stall every later DMA on that queue

                    def reshard_quarter(q):
                        if "nocc" in KBISECT:
                            return
                        w = QW[q]
                        ccr = cc_out[q][:].rearrange("k d w -> d k w")
                        half = N_CORES // 2
                        nc.gpsimd.dma_start(
                            ag[:, 0:half, QTOK[q]:QTOK[q] + w], ccr[:, 0:half, :]
                        )
                        nc.sync.dma_start(
                            ag[:, half:KE, QTOK[q]:QTOK[q] + w], ccr[:, half:KE, :]
                        )

                    def norm_subs(lq0, lqw, done, bst):
                        # transpose-free softmax divide for both heads; also
                        # combines the two row-tiled ctx partial sums (or
                        # plain-copies block 0's full-contraction result)
                        state = {}
                        pq = lqw // 4  # partitions used by the reshape DMAs

                        def s_copy():
                            CT0 = ctpool.tile([65, 512], BF16, tag="ct0", name="ct0")
                            CT1 = ctpool.tile([65, 512], BF16, tag="ct1", name="ct1")
                            nc.vector.tensor_copy(
                                CT0[:, :lqw], bst["PC0"][0:65, :lqw])
                            nc.vector.tensor_copy(
                                CT1[:, :lqw], bst["PC1"][0:65, :lqw])
                            state["CT0"] = CT0
                            state["CT1"] = CT1

                        def s_recip():
                            D1 = rpool.tile([128, 8], BF16, tag="d1", name="d1")
                            nc.scalar.dma_start(
                                D1[0:pq, 0:4], state["CT0"][64:65, :lqw])
                            nc.sync.dma_start(
                                D1[0:pq, 4:8], state["CT1"][64:65, :lqw])
                            R8 = rpool.tile([128, 8], BF16, tag="r8", name="r8")
                            with nc.allow_low_precision(
                                reason="bf16 recip of a bf16 denominator; "
                                "matches baseline numerics"
                            ):
                                nc.vector.reciprocal(
                                    R8[0:pq, :], D1[0:pq, :])
                            rr2d = rrdpool.tile([2, 512], BF16, tag="rr2d", name="rr2d")
                            nc.scalar.dma_start(rr2d[0:1, :lqw], R8[0:pq, 0:4])
                            nc.sync.dma_start(rr2d[1:2, :lqw], R8[0:pq, 4:8])
                            state["rr2d"] = rr2d

                        def s_bcast():
                            # partition-broadcast works with a DRAM source;
                            # two base-0 tiles (DVE needs equal input bases)
                            RB0 = ctpool.tile([64, 512], BF16, tag="rb0", name="rb0")
                            RB1 = ctpool.tile([64, 512], BF16, tag="rb1", name="rb1")
                            rr = state["rr2d"]
                            nc.scalar.dma_start(
                                RB0[:, :lqw],
                                rr[0:1, :lqw].to_broadcast((64, lqw)))
                            nc.sync.dma_start(
                                RB1[:, :lqw],
                                rr[1:2, :lqw].to_broadcast((64, lqw)))
                            state["RB0"] = RB0
                            state["RB1"] = RB1

                        def s_mul():
                            nc.vector.tensor_mul(
                                ctxTn[0:64, lq0:lq0 + lqw],
                                state["CT0"][0:64, :lqw],
                                state["RB0"][:, :lqw],
                            )
                            nc.vector.tensor_mul(
                                ctxTn[64:128, lq0:lq0 + lqw],
                                state["CT1"][0:64, :lqw],
                                state["RB1"][:, :lqw],
                            )
                            while cc_next[0] < 4 and done >= QEND[cc_next[0]]:
                                ship_quarter(cc_next[0])
                                cc_next[0] += 1

                        return [s_copy, s_recip, s_bcast, s_mul]

                    def flush_pend(n_keep):
                        while len(pend) > n_keep:
                            t, pb_, bst = pend.pop(0)
                            if bst["PC0"] is None:
                                for nm in ("PC0", "PC1"):
                                    bst[nm] = psx[0].tile(
                                        [128, 512], F32, tag=nm,
                                        name=f"{nm}_{bst['lq0']}",
                                    )
                            for h in range(2):
                                nc.tensor.matmul(
                                    bst[f"PC{h}"][0:65, :bst["lqw"]],
                                    vaug[:, t, 65 * h:65 * h + 65],
                                    pb_[:, h, :bst["lqw"]],
                                    start=(t == 0),
                                    stop=(t == NK - 1),
                                )

                    def attention_pass(blocks, tighten_tail=False):
                        for bi, (lq0, lqw) in enumerate(blocks):
                            is_last = tighten_tail and (lq0, lqw) == blocks[-1]
                            bst = {"PC0": None, "PC1": None,
                                   "lq0": lq0, "lqw": lqw}
                            sbs = [list(range(g, min(g + 2, NK)))
                                   for g in range(0, NK, 2)]
                            for gi, tl in enumerate(sbs):
                                for t in tl:
                                    SP = pssl[0].tile([128, 2, 512], F32, tag="sp", name="sp")
                                    score_pair(SP, t, lq0, lqw)
                                    PSb = pspool.tile([128, 2, 512], BF16, tag="psb", name="psb")
                                    nc.scalar.activation(
                                        PSb[:, :, :lqw], SP[:, :, :lqw],
                                        mybir.ActivationFunctionType.Exp,
                                        scale=SCALE,
                                    )
                                    pend.append((t, PSb, bst))
                                if gi >= 2:
                                    norm_step()
                                # keep>=4 before the first norm drip so a PC
                                # ring-slot reuse never precedes the s_copy
                                # that frees it
                                flush_pend(1 if (is_last and gi >= 8) else 4)
                            di = LQB.index((lq0, lqw))
                            norm_q.append({
                                "subs": norm_subs(lq0, lqw, LQB_DONE[di], bst),
                                "atomic": False,
                            })

                    # drain leftover phase-A items with phase-B score slots
                    for t in att0_left:
                        SP = spA.tile([128, 2, 512], F32, tag="sp0", name="sp0")
                        PSb = pspool.tile([128, 2, 512], BF16, tag="psb", name="psb")
                        score_pair(SP, t, 0, 512)
                        nc.scalar.activation(
                            PSb[:], SP[:],
                            mybir.ActivationFunctionType.Exp, scale=SCALE,
                        )
                        if att0_pend is not None:
                            att0_ctx(att0_pend)
                        att0_pend = (t, PSb)
                    if att0_pend is not None:
                        att0_ctx(att0_pend)
                        att0_pend = None

                    # block 0 came from phase A: queue its norm and start its
                    # DVE/DMA chain before block 1's score stream
                    norm_q.append({
                        "subs": norm_subs(
                            0, 512, QEND[0],
                            {"PC0": PC_A[0], "PC1": PC_A[1]},
                        ),
                        "atomic": False,
                    })
                    norm_step()  # s_copy consumes PC_A -> psc can close,
                    norm_step()  # freeing its 2 banks for the ctx partials
                    psc_cm.__exit__(None, None, None)
                    spA_cm.__exit__(None, None, None)
                    pss_cm = tc.tile_pool(name="ps_s", bufs=2, space="PSUM")
                    pssl[0] = pss_cm.__enter__()
                    psx_cm = tc.tile_pool(name="ps_x", bufs=1, space="PSUM")
                    psx[0] = psx_cm.__enter__()
                    attention_pass(LQB[1:], tighten_tail=True)
                    flush_pend(0)
                    while norm_q or cur_norm[0]:
                        norm_step()
                    psx_cm.__exit__(None, None, None)
                    pss_cm.__exit__(None, None, None)
                    # land the AllToAll results in SBUF: quarters 0-2 have
                    # long arrived (instant), only quarter 3's trigger waits
                    for q in range(4):
                        reshard_quarter(q)

                # ------------ Phase C: output projection ----------------------
                outTr = outT.rearrange("(k p) n -> p k n", p=128)
                with tc.tile_pool(name="ps_o", bufs=1, space="PSUM") as pso:
                    pos = [
                        pso.tile([128, SHARD], F32, tag=f"po{mE}", name=f"po{mE}")
                        for mE in range(KE)
                    ]
                    # stripes 0-2 (ag cols 0:192) depend only on AllToAlls
                    # 0-2 — the PE churns through them while AllToAll 3 is
                    # still in flight
                    for mE in range(KE):
                        for k in range(KE):
                            nc.tensor.matmul(
                                pos[mE][:, 0:192],
                                pw_sb[:, k, 128 * mE:128 * (mE + 1)],
                                ag[:, k, 0:192],
                                start=(k == 0),
                                stop=(k == KE - 1),
                            )
                    # stripe 3: mE-major so each output chunk's bias-add and
                    # store overlap the remaining chunks' matmuls
                    for mE in range(KE):
                        for k in range(KE):
                            nc.tensor.matmul(
                                pos[mE][:, 192:304],
                                pw_sb[:, k, 128 * mE:128 * (mE + 1)],
                                ag[:, k, 192:304],
                                start=(k == 0),
                                stop=(k == KE - 1),
                            )
                        nc.vector.tensor_scalar_add(
                            osb[:, mE, :], pos[mE][:], pbias[:, mE:mE + 1]
                        )
                        eng = nc.sync if mE % 2 == 0 else nc.gpsimd
                        eng.dma_start(outTr[:, mE, :], osb[:, mE, :])

    nc.compile()
    _NC_CACHE["nc"] = nc
    return nc


def _sigma():
    # kernel col -> original (padded) token index
    s = np.empty(LP, np.int64)
    for j in range(N_CORES):
        for q in range(4):
            s[QOFF[q] + QW[q] * j:QOFF[q] + QW[q] * (j + 1)] = (
                SHARD * j + QTOK[q] + np.arange(QW[q])
            )
    return s


def _prep_inputs(x, key_padding_mask, qkv_w, qkv_b, proj_w, proj_b, freqs_cos, freqs_sin):
    bf = ml_dtypes.bfloat16
    x = np.ascontiguousarray(np.asarray(x, np.float32))
    qkv_w = np.asarray(qkv_w, np.float32)
    qkv_b = np.asarray(qkv_b, np.float32)
    proj_w = np.asarray(proj_w, np.float32)
    proj_b = np.asarray(proj_b, np.float32)
    fc = np.asarray(freqs_cos, np.float32)  # [2304, 64]
    fs = np.asarray(freqs_sin, np.float32)
    mask = np.asarray(key_padding_mask)

    sig = _sigma()

    # chunk-major x in stripe-permuted token order: per NBLK block a
    # contiguous [128, KE*nw] slab with column order (k, n)
    xTf = x.T.astype(bf)  # [E, L]
    xH = np.concatenate(
        [
            xTf[:, sig[n0:n0 + nw]].reshape(KE, 128, nw).transpose(1, 0, 2).reshape(128, KE * nw)
            for (n0, nw) in NBLK
        ],
        axis=1,
    )
    xH = np.ascontiguousarray(xH)

    # rope tables + mask in kernel (permuted) token order
    valid = sig < L
    rot = valid & (sig >= 8)
    cosT = np.ones((64, LP), np.float32)
    cosT[:, rot] = fc.T[:, sig[rot] - 8]
    cos2 = np.concatenate([cosT, cosT], axis=0).astype(bf)

    sinT = np.zeros((64, LP), np.float32)
    sinT[:, rot] = fs.T[:, sig[rot] - 8]
    sinT[:32, :] *= -1.0  # sign of -x2 half folded into sin table
    sin2 = np.concatenate([sinT, sinT], axis=0).astype(bf)

    maskf = np.zeros((LP,), np.float32)
    maskf[valid] = mask.astype(np.float32)[sig[valid]]
    mskT = np.ascontiguousarray(maskf.reshape(NK, 128).T)  # [128, NK]

    # proj_w rows are consumed in natural head order (the quarter AllToAlls
    # deliver source cores' 128-row blocks in core order = head order)
    pwT = np.ascontiguousarray(proj_w.T).astype(bf)  # [d, e]
    permM = np.zeros((128, 128), np.float32)  # lhsT: permM[k, m]=1 iff k==swap(m)
    for m128 in range(128):
        swp = m128 + 32 if (m128 % 64) < 32 else m128 - 32
        permM[swp, m128] = 1.0
    permM = permM.astype(bf)
    pb2 = np.ascontiguousarray(proj_b.reshape(KE, 128).T)  # [128, KE]

    in_maps = []
    for c in range(N_CORES):
        h0, h1 = 2 * c, 2 * c + 1
        rows = []
        bias_rows = []
        for sec in range(3):  # q, k, v sections of qkv_w
            for h in (h0, h1):
                sl = slice(1024 * sec + 64 * h, 1024 * sec + 64 * h + 64)
                rows.append(qkv_w[sl])
                bias_rows.append(qkv_b[sl])
        Wc = np.concatenate(rows, axis=0)           # [384, 1024]
        bc = np.concatenate(bias_rows, axis=0)      # [384]
        WcT = Wc.T.astype(bf)  # [1024, 384]
        wH = np.ascontiguousarray(
            WcT.reshape(KE, 128, 3, 128).transpose(1, 2, 0, 3).reshape(128, 3 * KE * 128)
        )
        in_maps.append({
            "xT": xH,
            "wT": wH,
            "bqkv": np.ascontiguousarray(bc.reshape(3, 128).T),
            "cosT": cos2,
            "sinT": sin2,
            "mskT": mskT,
            "pwT": pwT,
            "pb": pb2,
            "perm": permM,
        })
    return in_maps


def _run(in_maps, trace=False):
    nc = _build()
    return run_bass_kernel_spmd(
        nc, in_maps, core_ids=list(range(N_CORES)), trace=trace
    )


def kernel(x, key_padding_mask, qkv_w, qkv_b, proj_w, proj_b, freqs_cos, freqs_sin):
    in_maps = _prep_inputs(
        x, key_padding_mask, qkv_w, qkv_b, proj_w, proj_b, freqs_cos, freqs_sin
    )
    res = _run(in_maps, trace=False)
    outT_full = np.concatenate(
        [res.results[c]["outT"] for c in range(N_CORES)], axis=1
    )  # [E, LP]; shard c's columns are original tokens [304c, 304c+304)
    return np.ascontiguousarray(outT_full[:, :L].T).astype(np.float32)
